# revision 1
# baseline (speedup 1.0000x reference)
"""Multi-head self-attention Bass/Tile kernel for Trainium2, 8 NeuronCores.

Problem: B=4, S=2048, D=1024, H=16 heads (HD=64), fp32, causal mask,
no padding.  y = softmax((xWq+bq)(xWk+bk)^T / 8 + mask) (xWv+bv) Wo + bo

Sharding (4-way batch x 2-way head-group):
  core c -> batch b = c//2, head group g = c%2 (heads 8g..8g+7).
  Each core computes its 8 heads' attention output and a PARTIAL
  out-projection y_partial = attn_out @ Wout[rows of its heads] (+ bout
  on g==0 cores only).  Host sums the two partials per batch.

Per-core kernel design (matmuls in float32r on the PE, fp32 elsewhere):
  Stage A: transpose x (PE transpose, fp32 exact) into x^T, then QKV
    projections in transposed layout.  Q^T/K^T are written to a DRAM
    scratch as [head, 128, S] with rows 64-127 ZERO so the attention
    matmuls can contract over the full 128 partitions (half-depth
    matmuls starve the PE's HAM activity monitor and the whole phase
    runs at 1.2 GHz instead of 2.4).
      V: [S, hd] natural SBUF-resident, per k-chunk, augmented with a
         one-hot column at 64+h -> softmax denominator for head h lands
         on psum partition 64+h of the AV matmul for free.
  Stage B1: per (head, q-tile of 512): scores^T[k,q] = K^T.T @ Q^T
    (128-deep, zero-padded), future blocks skipped entirely, diagonal
    blocks masked additively, exp on ACT (scale=0.125 folds 1/sqrt(64)),
    AV matmul accumulates attn_out^T[hd,q] (unnormalized) plus the
    denominator row; results parked in SBUF.
  Stage B2: one batched reciprocal over all denominators ([8,4,512]).
  Stage B3: PE-broadcast each reciprocal row, scale attn_out^T in place.
  Stage C: out projection from attn_out^T (hd on partitions).
"""

import sys

if "/opt/trn_rl_repo" not in sys.path:
    sys.path.insert(0, "/opt/trn_rl_repo")

import ml_dtypes
import numpy as np

import concourse.bass as bass
import concourse.mybir as mybir
import concourse.tile as tile
from concourse import bacc
from concourse.bass_utils import run_bass_kernel_spmd
from concourse.masks import make_identity

f32 = mybir.dt.float32
MM_DT = mybir.dt.float32r  # PE matmul dtype (float32r = ~2.7x faster than float32)

B, S, D, H = 4, 2048, 1024, 16
HD = D // H            # 64
P = 128
DC = D // P            # 8 contraction chunks for the projections
NPAIR = 4              # head pairs per core (8 local heads)
NST = S // 512         # 4 S-tiles of 512
NKC = S // P           # 16 k-chunks of 128
VW = HD + 8            # V_aug width: 64 V cols + 8 one-hot denominator cols
NEG = -1.0e30

BF16 = mybir.dt.bfloat16
AF = mybir.ActivationFunctionType
OP = mybir.AluOpType


def build_program():
    nc = bacc.Bacc("TRN2", target_bir_lowering=False, debug=False)

    xt_d = nc.dram_tensor("xT", [D, S], MM_DT, kind="ExternalInput")
    w_d = nc.dram_tensor("wqkv", [D, 3 * 512], MM_DT, kind="ExternalInput")
    b_d = nc.dram_tensor("bqkv", [3 * 512], f32, kind="ExternalInput")
    wo_d = nc.dram_tensor("wout", [512, D], MM_DT, kind="ExternalInput")
    bo_d = nc.dram_tensor("bout", [D], f32, kind="ExternalInput")
    cm_d = nc.dram_tensor("cmask", [4, P, 512], BF16, kind="ExternalInput")
    vm_d = nc.dram_tensor("vmask", [8, 8], MM_DT, kind="ExternalInput")
    sel_d = nc.dram_tensor("sel", [8, 8, HD], MM_DT, kind="ExternalInput")
    z_d = nc.dram_tensor("zeros", [S], MM_DT, kind="ExternalInput")
    y_d = nc.dram_tensor("y", [S, D], f32, kind="ExternalOutput")

    from contextlib import ExitStack

    with tile.TileContext(nc) as tc, ExitStack() as _lp:
        _lp.enter_context(
            nc.allow_low_precision(reason="float32r is 32-bit storage; rounding intended")
        )
        with tc.tile_pool(name="pers", bufs=1) as pers, \
             tc.tile_pool(name="consts", bufs=1) as consts, \
             tc.tile_pool(name="dram", bufs=1, space="DRAM") as dram:

            # DRAM scratch for pair-packed K^T, one tile per S-tile so stage
            # B's loads only depend on the producing S-tile's writes
            kz = [dram.tile([P, NPAIR, 512], MM_DT, tag=f"kz{j}", name=f"kz{j}")
                  for j in range(NST)]

            # ---- persistent activations ----
            # Q^T zero-padded per head: rows 0-63 real, 64-127 zero (shared)
            q_all = pers.tile([P, 8, S], MM_DT, tag="q")
            v_all = pers.tile([P, NKC, 8, VW], MM_DT, tag="v")

            # ---- constants ----
            sel_sb = consts.tile([72, 8, HD], MM_DT, tag="sel")
            bq_sb = consts.tile([P, 12], f32, tag="bq")
            vb_sb = consts.tile([P, 512], f32, tag="vb")
            cm_sb = consts.tile([P, 4, 512], BF16, tag="cm")

            # ================= Stage A: x^T + QKV projections =================
            with tc.tile_pool(name="wqkvp", bufs=1) as wqkvp, \
                 tc.tile_pool(name="xtp", bufs=2) as xtp, \
                 tc.tile_pool(name="qks", bufs=6) as qksp, \
                 tc.tile_pool(name="ps_mm", bufs=6, space="PSUM") as ps_mm:

                w_sb = wqkvp.tile([P, DC, 3 * 512], MM_DT, tag="wqkv")

                # x rows and weights first on the fast Sync queue; zero-fills
                # and small constants ride the GpSimd (SWDGE) queue.
                # first x^T tile on the GpSimd queue (starts instantly),
                # weights first on the fast Sync queue
                xt_r = xt_d.rearrange("(dc p) s -> p dc s", p=P)
                xt0 = xtp.tile([P, DC, 512], MM_DT, tag="xt")
                nc.sync.dma_start(out=xt0[:], in_=xt_r[:, :, 0:512])
                for dc in range(DC):
                    nc.sync.dma_start(
                        out=w_sb[:, dc, :], in_=w_d[dc * P : (dc + 1) * P, :]
                    )
                # small constants first (the first Q-projection needs bq_sb),
                # the big zero-fill of q_all's padding rows last
                nc.gpsimd.dma_start(out=bq_sb[:], in_=b_d.rearrange("(o p) -> p o", p=P))
                nc.gpsimd.dma_start(
                    out=vb_sb[:], in_=b_d[None, 1024:1536].to_broadcast([P, 512])
                )
                nc.gpsimd.dma_start(out=cm_sb[:], in_=cm_d.rearrange("m k q -> k m q"))
                nc.gpsimd.dma_start(out=sel_sb[64:72, :, :], in_=sel_d[:])


                xts = {0: xt0}

                def prefetch_xt(st):
                    t = xtp.tile([P, DC, 512], MM_DT, tag="xt", name=f"xt{st}")
                    nc.sync.dma_start(
                        out=t[:], in_=xt_r[:, :, st * 512 : (st + 1) * 512]
                    )
                    xts[st] = t

                for st in range(NST):
                    sl = slice(st * 512, (st + 1) * 512)
                    if st + 1 < NST:
                        prefetch_xt(st + 1)
                    xt = xts.pop(st)
                    # Q^T head-pair tiles -> zero-padded per-head SBUF layout
                    for pr in range(NPAIR):
                        mm = ps_mm.tile([P, 512], f32, tag="mm")
                        for dc in range(DC):
                            nc.tensor.matmul(
                                mm[:],
                                w_sb[:, dc, pr * P : (pr + 1) * P],
                                xt[:, dc, :],
                                start=(dc == 0),
                                stop=(dc == DC - 1),
                            )
                        bcol = bq_sb[:, pr : pr + 1]
                        # rows 64-127 are never multiplied by nonzero K (K^T is
                        # zero-padded); they only need to be finite, so the
                        # full-width copies double as the padding fill
                        nc.scalar.activation(
                            out=q_all[:, 2 * pr, sl],
                            in_=mm[:],
                            func=AF.Identity,
                            bias=bcol,
                        )
                        nc.vector.tensor_tensor(
                            q_all[0:HD, 2 * pr + 1, sl],
                            mm[HD:P, :],
                            bcol[HD:P].to_broadcast([HD, 512]),
                            OP.add,
                        )
                        nc.vector.tensor_copy(
                            out=q_all[64:P, 2 * pr + 1, sl], in_=mm[HD:P, :]
                        )
                    # K^T head-pair tiles -> pair-packed DRAM scratch
                    for pr in range(NPAIR):
                        mm = ps_mm.tile([P, 512], f32, tag="mm")
                        c0 = 512 + pr * P
                        for dc in range(DC):
                            nc.tensor.matmul(
                                mm[:],
                                w_sb[:, dc, c0 : c0 + P],
                                xt[:, dc, :],
                                start=(dc == 0),
                                stop=(dc == DC - 1),
                            )
                        qks = qksp.tile([P, 512], MM_DT, tag="qks")
                        nc.scalar.activation(
                            out=qks[:],
                            in_=mm[:],
                            func=AF.Identity,
                            bias=bq_sb[:, 4 + pr : 5 + pr],
                        )
                        nc.sync.dma_start(out=kz[st][:, pr, :], in_=qks[:])
                    # V: natural [S, hd] layout per 128-row chunk, all 8 heads
                    for sb in range(4):
                        mm = ps_mm.tile([P, 512], f32, tag="mm")
                        for dc in range(DC):
                            nc.tensor.matmul(
                                mm[:],
                                xt[:, dc, sb * P : (sb + 1) * P],
                                w_sb[:, dc, 1024:1536],
                                start=(dc == 0),
                                stop=(dc == DC - 1),
                            )
                        kc = st * 4 + sb
                        nc.vector.tensor_tensor(
                            v_all[:, kc, :, 0:HD],
                            mm[:].rearrange("p (h d) -> p h d", h=8),
                            vb_sb[:].rearrange("p (h d) -> p h d", h=8),
                            OP.add,
                        )


            # ================= Stage B: attention =================
            with tc.tile_pool(name="attnp", bufs=1) as attnp:
              attn_t = attnp.tile([P, NPAIR, S], MM_DT, tag="attn")
              den = attnp.tile([72, NST, 512], MM_DT, tag="den")
              # two alternating zero-padded K^T tiles (distinct tiles so their
              # top halves stay zero and deps do not serialize head-to-head)
              kta = attnp.tile([P, S], MM_DT, tag="kta")
              ktb = attnp.tile([P, S], MM_DT, tag="ktb")
              nc.gpsimd.dma_start(
                  out=kta[64:P, :], in_=z_d[None, :].to_broadcast([HD, S])
              )
              nc.gpsimd.dma_start(
                  out=ktb[64:P, :], in_=z_d[None, :].to_broadcast([HD, S])
              )
              wo_sb = attnp.tile([P, 4, D], MM_DT, tag="wout")
              for pc in range(4):
                  nc.gpsimd.dma_start(
                      out=wo_sb[:, pc, :], in_=wo_d[pc * P : (pc + 1) * P, :]
                  )
              bo_sb = attnp.tile([P, D], f32, tag="bo")
              nc.gpsimd.dma_start(
                  out=bo_sb[:], in_=bo_d[None, :].to_broadcast([P, D])
              )
              # one-hot denominator columns of V_aug: col 64+j = (j == h)
              for h in range(8):
                  nc.gpsimd.dma_start(
                      out=v_all[:, :, h, HD:VW],
                      in_=vm_d[None, None, h, :].to_broadcast([P, NKC, 8]),
                  )
              with tc.tile_pool(name="ppool", bufs=6) as ppool, \
                   tc.tile_pool(name="ps_s", bufs=3, space="PSUM") as ps_s, \
                   tc.tile_pool(name="ps_av", bufs=2, space="PSUM") as ps_av:
                  # ---- B1: unnormalized attention + denominators ----
                  for h in range(8):
                      pr, half = h // 2, h % 2
                      po = HD * half
                      kt_sb = kta if h % 2 == 0 else ktb
                      for st in range(NST):
                          nc.sync.dma_start(
                              out=kt_sb[0:HD, st * 512 : (st + 1) * 512],
                              in_=kz[st][po : po + HD, pr, :],
                          )
                      for qt in range(NST):
                          q0 = qt * 512
                          nk = 4 * qt + 4
                          av = ps_av.tile([72, 512], f32, tag="av")
                          for kc in range(nk):
                              sp = ps_s.tile([P, 512], f32, tag="sp")
                              nc.tensor.matmul(
                                  sp[:],
                                  kt_sb[:, kc * P : (kc + 1) * P],
                                  q_all[:, h, q0 : q0 + 512],
                                  start=True,
                                  stop=True,
                              )
                              m = kc - 4 * qt
                              if m >= 0:
                                  nc.vector.tensor_tensor(
                                      sp[:], sp[:], cm_sb[:, m, :], OP.add
                                  )
                              pt = ppool.tile([P, 512], MM_DT, tag="pt")
                              nc.scalar.activation(
                                  out=pt[:], in_=sp[:], func=AF.Exp, scale=0.125
                              )
                              nc.tensor.matmul(
                                  av[:],
                                  v_all[:, kc, h, :],
                                  pt[:],
                                  start=(kc == 0),
                                  stop=(kc == nk - 1),
                              )
                          # park unnormalized output + denominator in SBUF
                          nc.vector.tensor_copy(
                              out=attn_t[po : po + HD, pr, q0 : q0 + 512],
                              in_=av[0:HD, :],
                          )
                          if h == 0:
                              nc.vector.tensor_copy(
                                  out=den[64:72, qt, :], in_=av[64:72, :]
                              )
                          else:
                              nc.vector.tensor_tensor(
                                  den[64:72, qt, :],
                                  den[64:72, qt, :],
                                  av[64:72, :],
                                  OP.add,
                              )

              # ---- B2: batched reciprocals (split per qt so B3 can start
              # as soon as the first one lands) ----
              for qt in range(NST):
                  nc.vector.reciprocal(den[64:72, qt, :], den[64:72, qt, :])

              # ---- B3: broadcast reciprocals, normalize in place ----
              with tc.tile_pool(name="ps_r", bufs=2, space="PSUM") as ps_r:
                  for qt in range(NST):
                      q0 = qt * 512
                      for h in range(8):
                          pr, half = h // 2, h % 2
                          po = HD * half
                          rb = ps_r.tile([HD, 512], f32, tag="rb")
                          nc.tensor.matmul(
                              rb[:],
                              sel_sb[64:72, h, :],
                              den[64:72, qt, :],
                              start=True,
                              stop=True,
                          )
                          nc.vector.tensor_tensor(
                              attn_t[po : po + HD, pr, q0 : q0 + 512],
                              attn_t[po : po + HD, pr, q0 : q0 + 512],
                              rb[:],
                              OP.mult,
                          )

              # ================= Stage C: out projection =================
              with tc.tile_pool(name="ystage", bufs=3) as ystage, \
                   tc.tile_pool(name="ps_y", bufs=2, space="PSUM") as ps_y:

                  for qc in range(S // P):
                      q0 = qc * P
                      yt = ystage.tile([P, D], f32, tag="yt")
                      for nb in range(2):
                          yp = ps_y.tile([P, 512], f32, tag="yp")
                          for pc in range(4):
                              nc.tensor.matmul(
                                  yp[:],
                                  attn_t[:, pc, q0 : q0 + P],
                                  wo_sb[:, pc, nb * 512 : (nb + 1) * 512],
                                  start=(pc == 0),
                                  stop=(pc == 3),
                              )
                          nc.vector.tensor_tensor(
                              yt[:, nb * 512 : (nb + 1) * 512],
                              yp[:],
                              bo_sb[:, nb * 512 : (nb + 1) * 512],
                              OP.add,
                          )
                      nc.gpsimd.dma_start(out=y_d[q0 : q0 + P, :], in_=yt[:])

    nc.finalize()
    return nc


_NC = None


def _get_nc():
    global _NC
    if _NC is None:
        _NC = build_program()
    return _NC


def _shard_inputs(x, causal_mask, padding_mask, W_qkv, b_qkv, W_out, b_out):
    x = np.ascontiguousarray(np.asarray(x, dtype=np.float32))
    W_qkv = np.asarray(W_qkv, dtype=np.float32)
    b_qkv = np.asarray(b_qkv, dtype=np.float32)
    W_out = np.asarray(W_out, dtype=np.float32)
    b_out = np.asarray(b_out, dtype=np.float32)
    causal_mask = np.asarray(causal_mask)
    padding_mask = np.asarray(padding_mask)

    assert not padding_mask.any(), "kernel assumes no padding"
    # additive diagonal-band mask tiles [m, k, q]: scores^T[k0+k, q0+q] is
    # masked iff causal_mask[q0+q, k0+k] with k0 = q0 + 128*m
    cm = np.zeros((4, P, 512), dtype=np.float32)
    for m in range(4):
        cm[m] = np.where(causal_mask[0:512, m * P : (m + 1) * P].T, NEG, 0.0)
    cm = cm.astype(ml_dtypes.bfloat16)

    in_maps = []
    for c in range(8):
        b, g = c // 2, c % 2
        cols = slice(g * 512, (g + 1) * 512)
        w_slice = np.concatenate(
            [W_qkv[:, cols], W_qkv[:, 1024:2048][:, cols], W_qkv[:, 2048:3072][:, cols]],
            axis=1,
        )
        b_slice = np.concatenate(
            [b_qkv[cols], b_qkv[1024:2048][cols], b_qkv[2048:3072][cols]]
        )
        in_maps.append(
            {
                "xT": np.ascontiguousarray(x[b].T),
                "wqkv": np.ascontiguousarray(w_slice),
                "bqkv": np.ascontiguousarray(b_slice),
                "wout": np.ascontiguousarray(W_out[g * 512 : (g + 1) * 512, :]),
                "bout": b_out if g == 0 else np.zeros_like(b_out),
                "cmask": cm,
                "vmask": np.eye(8, dtype=np.float32),
                "sel": np.repeat(np.eye(8, dtype=np.float32)[:, :, None], HD, axis=2),
                "zeros": np.zeros((S,), dtype=np.float32),
            }
        )
    return in_maps


def _run(in_maps, **kwargs):
    nc = _get_nc()
    return run_bass_kernel_spmd(nc, in_maps, core_ids=list(range(8)), **kwargs)


def kernel(**inputs):
    in_maps = _shard_inputs(**inputs)
    res = _run(in_maps)
    out = np.empty((B, S, D), dtype=np.float32)
    for b in range(B):
        out[b] = res.results[2 * b]["y"] + res.results[2 * b + 1]["y"]
    return out


def kernel_traced(**inputs):
    """Like kernel() but with NTFF tracing; returns (out, BassKernelResults)."""
    in_maps = _shard_inputs(**inputs)
    res = _run(in_maps, trace=True)
    out = np.empty((B, S, D), dtype=np.float32)
    for b in range(B):
        out[b] = res.results[2 * b]["y"] + res.results[2 * b + 1]["y"]
    return out, res

